# revision 1
# baseline (speedup 1.0000x reference)
"""CRF Viterbi decode kernel for Trainium2 (Bass/Tile), 8-core batch-parallel.

Problem: sequences [64, 1024, 256] f32, transitions [256, 256] f32.
Output: one_hot(viterbi_tags) [64, 1024, 256] f32.

Sharding: data-parallel over batch; 8 sequences per core; transitions
replicated. Each core runs the full T=1024 sequential Viterbi scan for its
8 sequences, then backtracks and writes the one-hot output.

Per-core algorithm (all fp32, bit-exact vs the jax reference):
  forward step s (1..T-1), per (b, h in {0,1}):
    m = transT_h + alpha[b]         # PE: identity-copy matmul + selector matmul
    rowmax = reduce_max(m)          # DVE, free-dim reduce
    bp = max_index(m, rowmax)       # DVE, first-index tie semantics (== jax argmax)
    alpha' = rowmax + E_s           # transpose rowmax cols to rows (PE), add E (DVE)
  backtrack: tag_{s-1} = bp_s[b, tag_s] via scalar_tensor_tensor gather-by-sum
  one-hot: iota == tag, tensor_scalar is_equal.

Backpointer layout: bp_row_store[32*(s%4)+b, s>>2, c] (f16, exact for 0..255).
Tag layout: tags_store[32*(t%4)+b, t] (f32).
"""

import os
import numpy as np

import concourse.bass as bass
import concourse.bacc as bacc
import concourse.mybir as mybir
from concourse import tile
from concourse.masks import make_identity
from concourse.bass_utils import run_bass_kernel_spmd

F32 = mybir.dt.float32
F16 = mybir.dt.float16
U16 = mybir.dt.uint16
AX = mybir.AxisListType
OP = mybir.AluOpType

B_CORE = 8     # sequences per core
C = 256        # classes
NCORES = 8


def _emit_fwd_step(nc, pools, consts, s_mod4, alpha_in, alpha_out, e_tile, staging):
    """One Viterbi forward step; backpointers go to staging[32*(s%4)+b, c]."""
    ps_m, ps_t, sb = pools
    ident, identh, transT, sel = consts

    acol = sb["acol"].tile([128, 2, B_CORE], F32, name="acol")
    bpu = sb["bpu"].tile([128, 16, 8], U16, name="bpu")
    for b in range(B_CORE):
        for h in range(2):
            m = ps_m.tile([128, C], F32, tag="m", name="m")
            nc.tensor.matmul(m[:], lhsT=ident[:], rhs=transT[h][:],
                             start=True, stop=False)
            nc.tensor.matmul(m[:], lhsT=sel[:, b, :], rhs=alpha_in[:],
                             start=False, stop=True)
            nc.vector.reduce_max(acol[:, h, b : b + 1], m[:], axis=AX.X)
            nc.vector.max_index(
                out=bpu[:, h * 8 + b, :],
                in_max=acol[:, h, b : b + 1].to_broadcast((128, 8)),
                in_values=m[:],
            )
    # compact backpointers: [128, 16] u16 -> f16
    bpc = sb["bpc"].tile([128, 16], F16, name="bpc")
    nc.scalar.copy(bpc[:], bpu[:, :, 0])
    # transposes + stores
    amax = sb["amax"].tile([B_CORE, C], F32, name="amax")
    base = 32 * s_mod4
    for h in range(2):
        aT = ps_t.tile([B_CORE, 128], F32, tag=f"aT{h}", name=f"aT{h}")
        nc.tensor.transpose(aT[:], acol[:, h, :], ident[:])
        nc.scalar.copy(amax[:, h * 128 : (h + 1) * 128], aT[:])
        bT = ps_t.tile([B_CORE, 128], F16, tag=f"bT{h}", name=f"bT{h}")
        nc.tensor.transpose(bT[:], bpc[:, h * 8 : (h + 1) * 8], identh[:])
        nc.scalar.copy(staging[base : base + 8, h * 128 : (h + 1) * 128], bT[:])
    # alpha_out = amax + E_s
    nc.vector.tensor_tensor(out=alpha_out[:], in0=amax[:], in1=e_tile[:], op=OP.add)


def _viterbi_kernel(tc, seq, trans, out, T):
    nc = tc.nc
    with tc.tile_pool(name="const", bufs=1) as const_pool, \
         tc.tile_pool(name="store", bufs=1) as store_pool, \
         tc.tile_pool(name="e", bufs=4) as e_pool, \
         tc.tile_pool(name="stage", bufs=2) as stage_pool, \
         tc.tile_pool(name="acol", bufs=2) as acol_pool, \
         tc.tile_pool(name="bpu", bufs=2) as bpu_pool, \
         tc.tile_pool(name="bpc", bufs=2) as bpc_pool, \
         tc.tile_pool(name="amax", bufs=2) as amax_pool, \
         tc.tile_pool(name="oh", bufs=4) as oh_pool, \
         tc.tile_pool(name="psm", bufs=4, space="PSUM") as ps_m, \
         tc.tile_pool(name="pst", bufs=1, space="PSUM") as ps_t:

        sb = {"acol": acol_pool, "bpu": bpu_pool, "bpc": bpc_pool, "amax": amax_pool}

        # ---- constants ----
        ident = const_pool.tile([128, 128], F32)
        make_identity(nc, ident[:])
        identh = const_pool.tile([128, 128], F16)
        nc.vector.tensor_copy(identh[:], ident[:])
        # transT[h][p, j] = transitions[j, h*128+p] via PE transpose of blocks
        transT = [const_pool.tile([128, C], F32, tag=f"transT{h}", name=f"transT{h}")
                  for h in range(2)]
        trans_sb = [const_pool.tile([128, C], F32, tag=f"trans_sb{j}", name=f"trans_sb{j}")
                    for j in range(2)]
        for j in range(2):
            nc.sync.dma_start(trans_sb[j][:], trans[j * 128 : (j + 1) * 128, :])
        for h in range(2):
            for j in range(2):
                tb = ps_m.tile([128, 128], F32, tag="m", name="tb")
                nc.tensor.transpose(tb[:], trans_sb[j][:, h * 128 : (h + 1) * 128], ident[:])
                nc.scalar.copy(transT[h][:, j * 128 : (j + 1) * 128], tb[:])
        # sel[k, b, m] = (k == b)
        sel = const_pool.tile([B_CORE, B_CORE, 128], F32)
        part_idx = const_pool.tile([B_CORE, 1], F32)
        nc.gpsimd.iota(part_idx[:], pattern=[[0, 1]], base=0, channel_multiplier=1,
                       allow_small_or_imprecise_dtypes=True)
        bvals = const_pool.tile([B_CORE, B_CORE, 128], F32)
        nc.gpsimd.iota(bvals[:], pattern=[[1, B_CORE], [0, 128]], base=0,
                       channel_multiplier=0, allow_small_or_imprecise_dtypes=True)
        nc.vector.tensor_scalar(out=sel[:], in0=bvals[:], scalar1=part_idx[:],
                                scalar2=None, op0=OP.is_equal)
        # iota row 0..255 on all partitions
        iota_c = const_pool.tile([128, C], F32)
        nc.gpsimd.iota(iota_c[:], pattern=[[1, C]], base=0, channel_multiplier=0,
                       allow_small_or_imprecise_dtypes=True)

        # ---- state ----
        n_rows = (T + 3) >> 2
        bp_row_store = store_pool.tile([128, n_rows, C], F16)
        tags_store = store_pool.tile([128, T], F32)
        tags_row = store_pool.tile([B_CORE, T], F32)
        alpha_q = [store_pool.tile([B_CORE, C], F32, tag=f"alpha{q}", name=f"alpha{q}")
                   for q in range(4)]

        consts = (ident, identh, transT, sel)
        pools = (ps_m, ps_t, sb)

        # ---- init: alpha_0 = E_0 ----
        nc.sync.dma_start(alpha_q[0][:], seq[:, 0, :])

        # ---- peeled steps s = 1, 2, 3 (staging row 0) ----
        staging = stage_pool.tile([128, C], F16, name="staging")
        nc.gpsimd.memset(staging[:], 0)
        for s in range(1, min(4, T)):
            e = e_pool.tile([B_CORE, C], F32, name="e")
            nc.sync.dma_start(e[:], seq[:, s, :])
            _emit_fwd_step(nc, pools, consts, s % 4,
                           alpha_q[(s - 1) % 4], alpha_q[s % 4], e, staging)
        nc.sync.dma_start(bp_row_store[:, 0, :], staging[:])

        # ---- main loop: s = 4u + r, u in [1, T/4), r in [0, 4) ----
        if T > 4:
            with tc.For_i(1, T >> 2) as u:
                staging = stage_pool.tile([128, C], F16, name="staging")
                nc.gpsimd.memset(staging[:], 0)
                for r in range(4):
                    e = e_pool.tile([B_CORE, C], F32, name="e")
                    nc.sync.dma_start(e[:], seq[:, bass.ds(u * 4 + r, 1), :])
                    _emit_fwd_step(nc, pools, consts, r,
                                   alpha_q[(r + 3) % 4], alpha_q[r], e, staging)
                nc.sync.dma_start(bp_row_store[:, bass.ds(u, 1), :], staging[:])

        # ---- last tag: argmax of final alpha ----
        final_alpha = alpha_q[(T - 1) % 4]
        lt_max = store_pool.tile([B_CORE, 8], F32)
        lt_idx = store_pool.tile([B_CORE, 8], U16)
        nc.vector.max(lt_max[:], final_alpha[:])
        nc.vector.max_index(out=lt_idx[:], in_max=lt_max[:], in_values=final_alpha[:])
        lt_f = store_pool.tile([B_CORE, 1], F32)
        nc.scalar.copy(lt_f[:], lt_idx[:, 0:1])
        lbase = 32 * ((T - 1) % 4)
        nc.scalar.copy(tags_store[lbase : lbase + 8, T - 1 : T], lt_f[:])

        # ---- backtrack (static): tag_{s-1} = bp_s[b, tag_s] ----
        bt_scratch = store_pool.tile([B_CORE, C], F32)
        for s in range(T - 1, 0, -1):
            base = 32 * (s % 4)
            base2 = 32 * ((s - 1) % 4)
            nc.vector.scalar_tensor_tensor(
                out=bt_scratch[:],
                in0=iota_c[base : base + 8, :],
                scalar=tags_store[base : base + 8, s : s + 1],
                in1=bp_row_store[base : base + 8, s >> 2, :],
                op0=OP.is_equal,
                op1=OP.mult,
                accum_out=tags_store[base2 : base2 + 8, s - 1 : s],
            )

        # ---- harvest tags into [8, T] rows ----
        for r in range(4):
            cnt = len(range(r, T, 4))
            src = bass.AP(tensor=tags_store.tensor,
                          offset=tags_store[32 * r : 32 * r + 8, r : r + 1].offset,
                          ap=[[T, 8], [4, cnt]])
            dst = bass.AP(tensor=tags_row.tensor,
                          offset=tags_row[:, r : r + 1].offset,
                          ap=[[T, 8], [4, cnt]])
            nc.sync.dma_start(out=dst, in_=src)

        # ---- one-hot output ----
        n_chunks = (T + 127) // 128
        for k in range(n_chunks):
            w_ = min(128, T - k * 128)
            tagT = ps_m.tile([128, B_CORE], F32, tag="m", name="tagT")
            nc.tensor.transpose(tagT[:w_, :], tags_row[:, k * 128 : k * 128 + w_],
                                ident[:B_CORE, :B_CORE])
            for b in range(B_CORE):
                oh = oh_pool.tile([128, C], F32, name="oh")
                nc.vector.tensor_scalar(out=oh[:w_, :], in0=iota_c[:w_, :],
                                        scalar1=tagT[:w_, b : b + 1], scalar2=None,
                                        op0=OP.is_equal)
                nc.sync.dma_start(out[b, k * 128 : k * 128 + w_, :], oh[:w_, :])


def _build(T=1024):
    nc = bacc.Bacc("TRN2", target_bir_lowering=False, debug=False)
    seq = nc.dram_tensor("seq", [B_CORE, T, C], F32, kind="ExternalInput")
    trans = nc.dram_tensor("trans", [C, C], F32, kind="ExternalInput")
    out = nc.dram_tensor("out", [B_CORE, T, C], F32, kind="ExternalOutput")
    with tile.TileContext(nc) as tc:
        _viterbi_kernel(tc, seq.ap(), trans.ap(), out.ap(), T)
    nc.compile()
    return nc


def kernel(sequences: np.ndarray, transitions: np.ndarray) -> np.ndarray:
    B, T, C_ = sequences.shape
    assert C_ == C and B == B_CORE * NCORES
    nc = _build(T)
    in_maps = [
        {
            "seq": np.ascontiguousarray(sequences[i * B_CORE : (i + 1) * B_CORE]),
            "trans": np.ascontiguousarray(transitions),
        }
        for i in range(NCORES)
    ]
    trace = os.environ.get("CRF_TRACE", "0") == "1"
    res = None
    last_exc = None
    for attempt in range(4):
        try:
            res = run_bass_kernel_spmd(nc, in_maps, core_ids=list(range(NCORES)),
                                       trace=trace)
            break
        except Exception as e:  # transient device-unrecoverable flakes
            last_exc = e
    if res is None:
        raise last_exc
    if trace and res.exec_time_ns is not None:
        print(f"HW exec time: {res.exec_time_ns} ns")
    return np.concatenate([r["out"] for r in res.results], axis=0)

